# revision 25
# baseline (speedup 1.0000x reference)
"""Causal self-attention kernel for 8 TRN2 NeuronCores (v2, bf16).

Sharding: core = b*4 + g  (b = batch 0..1, g = head-group 0..3, 4 heads each).
Each core computes, for its batch b and its 4 heads:
  qkv projection -> per-head causal attention (softmax without max-subtraction,
  scores are bounded ~N(0,1)) -> partial output projection over its 256
  attn columns.  Host sums the 4 per-batch partials and adds the bias.

v2 changes vs the fp32r baseline:
  * bf16 operands end-to-end (fp32 PSUM accumulation): halves input DMA,
    enables FWL weight loads and DVE 2x modes.  Measured rel err ~5.6e-3.
  * scores for a head PAIR run concurrently as 64x128 row-tiles (T0/T8),
    halving score matmul time.
  * per-j-tile diagonal skips (exp starts at the first live query column).
  * softmax normalization: DVE reciprocal_approx_fast (~5x faster than
    reciprocal) on a packed [97,512] rowsum tile.

On-device layout (per core):
  xT     [E=1024, S=2048]  bf16 host-pretransposed x[b].T
  wqkvT  [E, F=768]        bf16 host-built [Wq_g; Wk_g; Wv_g].T
  woutT  [256, E]          bf16 host-built w_out[:, 256g:256g+256].T
  mask   [128, 128]        bf16 causal triangle (col >= row)
  out    [S, E]            f32 partial output (pre-bias)
"""

import os

import numpy as np

_B, _S, _E = 2, 2048, 1024
_H, _D = 16, 64
_F = 768  # per-core qkv rows: 4 heads * 3 * 64
_P = 128

# stash of the last profiled exec time (ns), for test harnesses
LAST_EXEC_TIME_NS = None

_PROGRAM_CACHE = {}


def _build_program(S=_S):
    import concourse.bacc as bacc
    import concourse.mybir as mybir
    import concourse.tile as tile

    f32 = mybir.dt.float32
    f32r = mybir.dt.float32r
    bf16 = mybir.dt.bfloat16
    Exp = mybir.ActivationFunctionType.Exp

    P = _P
    E, F = _E, _F
    NCH = E // P          # 8 contraction chunks for the projections
    NSB = S // 512        # s-blocks of 512
    NIB = S // 512        # i-blocks (attention query blocks)

    nc = bacc.Bacc("TRN2", target_bir_lowering=False, debug=False)

    xT = nc.declare_dram_parameter("xT", [E, S], bf16, isOutput=False)
    wqkvT = nc.declare_dram_parameter("wqkvT", [E, F], bf16, isOutput=False)
    woutT = nc.declare_dram_parameter("woutT", [256, E], bf16, isOutput=False)
    maskd = nc.declare_dram_parameter("mask", [P, 128], bf16, isOutput=False)
    outd = nc.declare_dram_parameter("out", [S, E], bf16, isOutput=True)

    x3 = xT[:].rearrange("(ko p) s -> p ko s", p=P)      # [128, 8, S]
    w3 = wqkvT[:].rearrange("(ko p) f -> p ko f", p=P)   # [128, 8, 768]
    wo3 = woutT[:].rearrange("(c p) e -> p c e", p=P)    # [128, 2, 1024]

    with tile.TileContext(nc) as tc:
        with (
            tc.tile_pool(name="consts", bufs=1) as consts,
            tc.tile_pool(name="xpool", bufs=2) as xpool,
            tc.tile_pool(name="qkpool", bufs=1) as qkpool,
            tc.tile_pool(name="vpool", bufs=1) as vpool,
            tc.tile_pool(name="atpool", bufs=1) as atpool,
            tc.tile_pool(name="probs", bufs=3) as probs,
            tc.tile_pool(name="small", bufs=2) as small,
            tc.tile_pool(name="outpool", bufs=3) as outpool,
            tc.tile_pool(name="psum", bufs=2, space="PSUM") as psum,
        ):
            # ---- constants ----
            # per-chunk weight tiles: a consumer waits only on its own chunk
            w_t = [consts.tile([P, F], bf16, name=f"w{ch}") for ch in range(NCH)]
            wo_sb = consts.tile([P, 2, E], bf16)
            mask_sb = consts.tile([P, 1, 128], bf16)
            ones97 = consts.tile([97, 64], bf16)
            ones_bf = consts.tile([P, 1, 1], bf16)

            nc.vector.memset(ones97[:], 1.0)
            nc.vector.memset(ones_bf[:], 1.0)

            # per-s-block persistent activations, split per head-pair /
            # per key-tile so consumers only wait on the producer they need.
            # qk_t[s][hp][:, f, :]: f=0 q, f=1 k; partitions 0:64 = even head
            # of pair hp, 64:128 = odd head
            qk_t = [[qkpool.tile([P, 2, 512], bf16, name=f"qk{s}_{hp}")
                     for hp in range(2)] for s in range(NSB)]
            v_t = [[vpool.tile([P, 4 * 65], bf16, name=f"v{s}_{st}")
                    for st in range(4)] for s in range(NSB)]
            at_t = [atpool.tile([P, 2, 512], bf16, name=f"at{s}") for s in range(NIB)]
            v4 = [[v_t[s][st].rearrange("p (h e) -> p h e", h=4) for st in range(4)]
                  for s in range(NSB)]

            # ones columns of v_aug (row-sum trick for softmax denominators)
            for s in range(NSB):
                for st in range(4):
                    nc.vector.tensor_copy(
                        v_t[s][st][:, 64::65],
                        ones_bf[:].to_broadcast((P, 1, 4)),
                    )

            # filler queue: (pe_cost_ns, deadline_ib, fn).  Items are popped
            # FIFO but paced by a per-batch PE-time budget so deferred work
            # (projections, out-proj, normalize) spreads uniformly across the
            # ACT-bound attention instead of draining greedily early.  At each
            # ib boundary, items whose deadline has arrived are flushed so
            # emission order still precedes their consumers.
            filler = []
            credit = [0.0]

            def drain_budget(ns):
                credit[0] += ns
                while filler and credit[0] > 0:
                    cost, _, fn = filler.pop(0)
                    fn()
                    credit[0] -= cost

            def flush_due(ib):
                keep = []
                for item in filler:
                    if item[1] <= ib:
                        item[2]()
                    else:
                        keep.append(item)
                filler[:] = keep

            def emit_proj(sbk, enqueue):
                """qkv projection for s-block sbk; enqueue=True drips the
                matmul groups through the filler queue so they pack into
                attention's ACT-bound gaps."""
                s0 = 512 * sbk
                xt = [xpool.tile([P, 512], bf16, tag=f"xt{ch}", name=f"xt{sbk}_{ch}")
                      for ch in range(NCH)]
                for ch in range(NCH):
                    # gpsimd = software DGE queue; keeps DMA issue off the
                    # scalar engine whose cycles the softmax Exp needs
                    nc.gpsimd.dma_start(xt[ch][:], x3[:, ch, s0:s0 + 512])
                    if sbk == 0:
                        weng = nc.sync if ch % 2 == 0 else nc.scalar
                        weng.dma_start(w_t[ch][:], w3[:, ch])

                def qk_group(ft, sbk=sbk, xt=xt):
                    qkps = psum.tile([P, 512], f32, tag="acc", bufs=2,
                                     name=f"qkps{sbk}_{ft}")
                    for ch in range(NCH):
                        nc.tensor.matmul(
                            qkps[:],
                            lhsT=w_t[ch][:, 128 * ft:128 * (ft + 1)],
                            rhs=xt[ch][:],
                            start=(ch == 0), stop=(ch == NCH - 1),
                            skip_group_check=True,
                        )
                    nc.any.tensor_copy(qk_t[sbk][ft % 2][:, ft // 2, :],
                                       qkps[:])

                def v_group(st, sbk=sbk, xt=xt):
                    vps = psum.tile([P, 256], f32, tag="acc", bufs=2,
                                    name=f"vps{sbk}_{st}")
                    for ch in range(NCH):
                        nc.tensor.matmul(
                            vps[:],
                            lhsT=xt[ch][:, 128 * st:128 * (st + 1)],
                            rhs=w_t[ch][:, 512:768],
                            start=(ch == 0), stop=(ch == NCH - 1),
                            skip_group_check=True,
                        )
                    nc.any.tensor_copy(
                        v4[sbk][st][:, :, 0:64],
                        vps.rearrange("p (h e) -> p h e", h=4),
                    )

                # pair-0 q/k first so attention(sbk, hp=0) unblocks early
                order = [(qk_group, 0), (qk_group, 2), (v_group, 0), (v_group, 1),
                         (v_group, 2), (v_group, 3), (qk_group, 1), (qk_group, 3)]
                for fn, i in order:
                    if enqueue:
                        # proj for s-block sbk must be fully emitted before
                        # attention(ib=sbk) scores: deadline = sbk - 1
                        cost = 1710 if fn is qk_group else 855
                        filler.append((cost, sbk - 1, lambda fn=fn, i=i: fn(i)))
                    else:
                        fn(i)

            def enqueue_normalize(ib, rs_ib):
                def recip_item(ib=ib, rs_ib=rs_ib):
                    rs_inv = small.tile([97, 512], f32, tag="rsi", name=f"rsi{ib}")
                    nc.vector.reciprocal_approx_fast(rs_inv[:], rs_ib[:])
                    rs_inv_b = small.tile([97, 512], bf16, tag="rsib",
                                          name=f"rsib{ib}")
                    nc.vector.tensor_copy(rs_inv_b[:], rs_inv[:])
                    _state[ib] = rs_inv_b

                def norm_head(h, ib=ib):
                    rs_inv = _state[ib]
                    po = 64 * (h % 2)
                    hp = h // 2
                    bcps = psum.tile([64, 512], f32, tag="acc", bufs=2,
                                     name=f"bcps{h}_{ib}")
                    nc.tensor.matmul(
                        bcps[:], lhsT=ones97[32 * h:32 * h + 1, :],
                        rhs=rs_inv[32 * h:32 * h + 1, :],
                        start=True, stop=True,
                        skip_group_check=True,
                        tile_position=(32 * h, 0),
                    )
                    nc.vector.tensor_mul(
                        at_t[ib][po:po + 64, hp, :],
                        at_t[ib][po:po + 64, hp, :], bcps[:]
                    )

                # rs pool bufs=2: recip(ib) must emit before rs(ib+2) memset
                filler.append((100, ib + 1, recip_item))
                for h in range(4):
                    filler.append((215, ib + 1, lambda h=h: norm_head(h)))

            def enqueue_outproj(ib):
                def op_item(its, ec, ib=ib):
                    it = 4 * ib + its
                    key = ("ot", it)
                    if ec == 0:
                        _state[key] = outpool.tile([P, E], bf16, tag="ot",
                                                   name=f"ot{it}")
                    ot = _state[key]
                    ops = psum.tile([P, 512], f32, tag="acc", bufs=2,
                                    name=f"ops{it}_{ec}")
                    for c in range(2):
                        nc.tensor.matmul(
                            ops[:],
                            lhsT=at_t[ib][:, c, 128 * its:128 * (its + 1)],
                            rhs=wo_sb[:, c, 512 * ec:512 * (ec + 1)],
                            start=(c == 0), stop=(c == 1),
                            skip_group_check=True,
                        )
                    nc.vector.tensor_copy(ot[:, 512 * ec:512 * (ec + 1)],
                                          ops[:])
                    if ec == 1:
                        nc.sync.dma_start(outd[128 * it:128 * (it + 1), :], ot[:])

                for its in range(4):
                    for ec in range(2):
                        filler.append((427, 99, lambda its=its, ec=ec: op_item(its, ec)))

            _state = {}
            emit_proj(0, enqueue=False)
            nc.sync.dma_start(mask_sb[:, 0, :], maskd[:])
            nc.sync.dma_start(wo_sb[:], wo3[:])

            # ---- attention: (ib, head-pair) sweeps, software-pipelined.
            # Per j-tile: paired scores (64x128 row tiles T0+T8 run
            # concurrently), one Exp evacuating both heads, causal mask mul
            # on the diagonal, then AV accumulation per head.  The AV of
            # j-tile pair k runs while ACT computes exp of pair k+1.
            def emit_scores(ib, hp, bat, pbr):
                # one 4-bank score tile per batch of 2 j-tiles: f = 2*jj+par.
                # T0/T8 row-tile pairs land in different banks and run
                # concurrently; non-diagonal batches evacuate all 4 banks
                # with a single Exp (halves ACT instruction+semaphore
                # overhead), diagonal batches use one Exp per j-tile.
                sc = psum.tile([P, 4, 512], f32, tag="sc", bufs=1,
                               name=f"sc{ib}_{hp}_{bat}")
                pb = probs.tile([P, 4, 512], bf16, tag="pb",
                                name=f"pb{ib}_{hp}_{bat}")
                pbr[0] = pb
                diag = 2 * bat - 4 * ib >= 0
                for jj in range(2):
                    jt = 2 * bat + jj
                    t = jt - 4 * ib
                    e0 = 128 * t if t >= 0 else 0
                    lt = jt % 4
                    sb = jt // 4
                    for par in range(2):
                        nc.tensor.matmul(
                            sc[:, 2 * jj + par, e0:],
                            lhsT=qk_t[sb][hp][64 * par:64 * par + 64, 1,
                                              128 * lt:128 * (lt + 1)],
                            rhs=qk_t[ib][hp][64 * par:64 * par + 64, 0, e0:],
                            start=True, stop=True, skip_group_check=True,
                        )
                    if diag:
                        # probs = exp(scores / sqrt(D)); no max-subtraction
                        nc.scalar.activation(pb[:, 2 * jj:2 * jj + 2, e0:],
                                             sc[:, 2 * jj:2 * jj + 2, e0:],
                                             Exp, scale=0.125)
                        # causal triangle on the partially-masked 128 columns
                        nc.vector.tensor_mul(
                            pb[:, 2 * jj:2 * jj + 2, e0:e0 + 128],
                            pb[:, 2 * jj:2 * jj + 2, e0:e0 + 128],
                            mask_sb[:, 0:1, :].to_broadcast((P, 2, 128)),
                        )
                if not diag:
                    nc.scalar.activation(pb[:], sc[:], Exp, scale=0.125)

            def emit_av(ib, hp, bat, pbr, atp2, njt):
                pb = pbr[0]
                for jj in range(2):
                    jt = 2 * bat + jj
                    t = jt - 4 * ib
                    c0 = 128 * t if t > 0 else 0
                    for par in range(2):
                        nc.tensor.matmul(
                            atp2[par][:, c0:],
                            lhsT=v4[jt // 4][jt % 4][:, 2 * hp + par, :],
                            rhs=pb[:, 2 * jj + par, c0:],
                            start=(jt == 0), stop=(jt == njt - 1),
                            skip_group_check=True,
                        )

            def evac_pair(ib, hp, atp2, rs_ib):
                for par in range(2):
                    h = 2 * hp + par
                    nc.vector.tensor_copy(rs_ib[32 * h:32 * h + 1, :],
                                          atp2[par][64:65, :])
                    nc.any.tensor_copy(
                        at_t[ib][64 * par:64 * par + 64, hp, :],
                        atp2[par][0:64, :],
                    )

            # per-batch filler PE budget: total deferred PE work (~48us)
            # spread over the 40 j-tile-pair batches
            BAT_BUDGET = 1000

            pend = None  # (ib, hp, bat, pb2, atp2, njt) awaiting AV
            for ib in range(NIB):
                if ib + 1 < NSB:
                    emit_proj(ib + 1, enqueue=True)
                rs_ib = small.tile([97, 512], f32, tag="rs", name=f"rs{ib}")
                nc.vector.memset(rs_ib[:], 1.0)
                _state[("rs", ib)] = rs_ib
                for hp in range(2):
                    njt = 4 * (ib + 1)
                    atp2 = [psum.tile([65, 512], f32, tag="at", bufs=2,
                                      name=f"atps{ib}_{hp}_{par}")
                            for par in range(2)]
                    for bat in range(njt // 2):
                        pb2 = [None]
                        emit_scores(ib, hp, bat, pb2)
                        if pend is not None:
                            emit_av(*pend)
                            if pend[0] != ib or pend[1] != hp:
                                # previous pair finished: evacuate + enqueue
                                p_ib, p_hp = pend[0], pend[1]
                                evac_pair(p_ib, p_hp, pend[4], _state[("rs", p_ib)])
                                if p_hp == 1:
                                    enqueue_normalize(p_ib, _state[("rs", p_ib)])
                                    enqueue_outproj(p_ib)
                        pend = (ib, hp, bat, pb2, atp2, njt)
                        drain_budget(BAT_BUDGET)
                flush_due(ib)
            # tail
            emit_av(*pend)
            evac_pair(pend[0], pend[1], pend[4], _state[("rs", pend[0])])
            enqueue_normalize(pend[0], _state[("rs", pend[0])])
            enqueue_outproj(pend[0])
            while filler:
                filler.pop(0)[2]()

    nc.compile()
    return nc


def _get_program(S=_S):
    if S not in _PROGRAM_CACHE:
        _PROGRAM_CACHE[S] = _build_program(S)
    return _PROGRAM_CACHE[S]


def _make_mask():
    import ml_dtypes
    pp = np.arange(_P)[:, None]
    cc = np.arange(128)[None, :]
    return (cc >= pp).astype(ml_dtypes.bfloat16)


def make_in_maps(x, w_qkv, w_out):
    import ml_dtypes
    bf16 = ml_dtypes.bfloat16
    x = np.asarray(x, np.float32)
    w_qkv = np.asarray(w_qkv, np.float32)
    w_out = np.asarray(w_out, np.float32)
    E = _E
    mask = _make_mask()
    xTs = [np.ascontiguousarray(x[b].T).astype(bf16) for b in range(_B)]
    wqs, wos = [], []
    for g in range(4):
        W = np.concatenate(
            [
                w_qkv[256 * g:256 * (g + 1)],
                w_qkv[E + 256 * g:E + 256 * (g + 1)],
                w_qkv[2 * E + 256 * g:2 * E + 256 * (g + 1)],
            ],
            axis=0,
        )  # [768, E]
        wqs.append(np.ascontiguousarray(W.T).astype(bf16))          # [E, 768]
        wos.append(np.ascontiguousarray(
            w_out[:, 256 * g:256 * (g + 1)].T).astype(bf16))        # [256, E]
    in_maps = []
    for core in range(8):
        b, g = core // 4, core % 4
        in_maps.append(
            {"xT": xTs[b], "wqkvT": wqs[g], "woutT": wos[g], "mask": mask}
        )
    return in_maps


LAST_TRACE_DIR = None


def _enable_jax_compile_cache():
    try:
        import jax

        jax.config.update("jax_compilation_cache_dir", "/tmp/jax_cache")
        jax.config.update("jax_persistent_cache_min_compile_time_secs", 0.0)
        jax.config.update("jax_persistent_cache_min_entry_size_bytes", -1)
    except Exception:
        pass


def kernel(x, w_qkv, w_out, b_out):
    global LAST_EXEC_TIME_NS, LAST_TRACE_DIR
    from concourse.bass_utils import run_bass_kernel_spmd

    _enable_jax_compile_cache()
    b_out = np.asarray(b_out, np.float32)
    in_maps = make_in_maps(x, w_qkv, w_out)
    nc = _get_program()
    trace = bool(int(os.environ.get("BASS_PROFILE", "0")))
    tmpdir = None
    if trace:
        import tempfile

        tmpdir = tempfile.mkdtemp(prefix="bass_trace_")
        LAST_TRACE_DIR = tmpdir
    res = run_bass_kernel_spmd(
        nc, in_maps, core_ids=list(range(8)), trace=trace, tmpdir=tmpdir
    )
    LAST_EXEC_TIME_NS = res.exec_time_ns
    out = np.zeros((_B, _S, _E), np.float32)
    for core in range(8):
        out[core // 4] += np.asarray(res.results[core]["out"], np.float32)
    out += b_out[None, None, :]
    return out


# revision 26
# speedup vs baseline: 1.0369x; 1.0369x over previous
"""Causal self-attention kernel for 8 TRN2 NeuronCores (v2, bf16).

Sharding: core = b*4 + g  (b = batch 0..1, g = head-group 0..3, 4 heads each).
Each core computes, for its batch b and its 4 heads:
  qkv projection -> per-head causal attention (softmax without max-subtraction,
  scores are bounded ~N(0,1)) -> partial output projection over its 256
  attn columns.  Host sums the 4 per-batch partials and adds the bias.

v2 changes vs the fp32r baseline:
  * bf16 operands end-to-end (fp32 PSUM accumulation): halves input DMA,
    enables FWL weight loads and DVE 2x modes.  Measured rel err ~5.6e-3.
  * scores for a head PAIR run concurrently as 64x128 row-tiles (T0/T8),
    halving score matmul time.
  * per-j-tile diagonal skips (exp starts at the first live query column).
  * softmax normalization: DVE reciprocal_approx_fast (~5x faster than
    reciprocal) on a packed [97,512] rowsum tile.

On-device layout (per core):
  xT     [E=1024, S=2048]  bf16 host-pretransposed x[b].T
  wqkvT  [E, F=768]        bf16 host-built [Wq_g; Wk_g; Wv_g].T
  woutT  [256, E]          bf16 host-built w_out[:, 256g:256g+256].T
  mask   [128, 128]        bf16 causal triangle (col >= row)
  out    [S, E]            f32 partial output (pre-bias)
"""

import os

import numpy as np

_B, _S, _E = 2, 2048, 1024
_H, _D = 16, 64
_F = 768  # per-core qkv rows: 4 heads * 3 * 64
_P = 128

# stash of the last profiled exec time (ns), for test harnesses
LAST_EXEC_TIME_NS = None

_PROGRAM_CACHE = {}


def _build_program(S=_S):
    import concourse.bacc as bacc
    import concourse.mybir as mybir
    import concourse.tile as tile

    f32 = mybir.dt.float32
    f32r = mybir.dt.float32r
    bf16 = mybir.dt.bfloat16
    Exp = mybir.ActivationFunctionType.Exp

    P = _P
    E, F = _E, _F
    NCH = E // P          # 8 contraction chunks for the projections
    NSB = S // 512        # s-blocks of 512
    NIB = S // 512        # i-blocks (attention query blocks)

    nc = bacc.Bacc("TRN2", target_bir_lowering=False, debug=False)

    xT = nc.declare_dram_parameter("xT", [E, S], bf16, isOutput=False)
    wqkvT = nc.declare_dram_parameter("wqkvT", [E, F], bf16, isOutput=False)
    woutT = nc.declare_dram_parameter("woutT", [256, E], bf16, isOutput=False)
    maskd = nc.declare_dram_parameter("mask", [P, 128], bf16, isOutput=False)
    outd = nc.declare_dram_parameter("out", [S, E], bf16, isOutput=True)

    x3 = xT[:].rearrange("(ko p) s -> p ko s", p=P)      # [128, 8, S]
    w3 = wqkvT[:].rearrange("(ko p) f -> p ko f", p=P)   # [128, 8, 768]
    wo3 = woutT[:].rearrange("(c p) e -> p c e", p=P)    # [128, 2, 1024]

    with tile.TileContext(nc) as tc:
        with (
            tc.tile_pool(name="consts", bufs=1) as consts,
            tc.tile_pool(name="xpool", bufs=2) as xpool,
            tc.tile_pool(name="qkpool", bufs=1) as qkpool,
            tc.tile_pool(name="vpool", bufs=1) as vpool,
            tc.tile_pool(name="atpool", bufs=1) as atpool,
            tc.tile_pool(name="probs", bufs=3) as probs,
            tc.tile_pool(name="small", bufs=2) as small,
            tc.tile_pool(name="outpool", bufs=3) as outpool,
            tc.tile_pool(name="psum", bufs=2, space="PSUM") as psum,
        ):
            # ---- constants ----
            # per-chunk weight tiles: a consumer waits only on its own chunk
            w_t = [consts.tile([P, F], bf16, name=f"w{ch}") for ch in range(NCH)]
            wo_sb = consts.tile([P, 2, E], bf16)
            mask_sb = consts.tile([P, 1, 128], bf16)
            ones97 = consts.tile([97, 64], bf16)
            ones_bf = consts.tile([P, 1, 1], bf16)

            nc.vector.memset(ones97[:], 1.0)
            nc.vector.memset(ones_bf[:], 1.0)

            # per-s-block persistent activations, split per head-pair /
            # per key-tile so consumers only wait on the producer they need.
            # qk_t[s][hp][:, f, :]: f=0 q, f=1 k; partitions 0:64 = even head
            # of pair hp, 64:128 = odd head
            qk_t = [[qkpool.tile([P, 2, 512], bf16, name=f"qk{s}_{hp}")
                     for hp in range(2)] for s in range(NSB)]
            v_t = [[vpool.tile([P, 4 * 65], bf16, name=f"v{s}_{st}")
                    for st in range(4)] for s in range(NSB)]
            at_t = [atpool.tile([P, 2, 512], bf16, name=f"at{s}") for s in range(NIB)]
            v4 = [[v_t[s][st].rearrange("p (h e) -> p h e", h=4) for st in range(4)]
                  for s in range(NSB)]

            # ones columns of v_aug (row-sum trick for softmax denominators)
            for s in range(NSB):
                for st in range(4):
                    nc.vector.tensor_copy(
                        v_t[s][st][:, 64::65],
                        ones_bf[:].to_broadcast((P, 1, 4)),
                    )

            # filler queue: (pe_cost_ns, deadline_ib, fn).  Items are popped
            # FIFO but paced by a per-batch PE-time budget so deferred work
            # (projections, out-proj, normalize) spreads uniformly across the
            # ACT-bound attention instead of draining greedily early.  At each
            # ib boundary, items whose deadline has arrived are flushed so
            # emission order still precedes their consumers.
            filler = []
            credit = [0.0]

            def drain_budget(ns):
                credit[0] += ns
                while filler and credit[0] > 0:
                    cost, _, fn = filler.pop(0)
                    fn()
                    credit[0] -= cost

            def flush_due(ib):
                keep = []
                for item in filler:
                    if item[1] <= ib:
                        item[2]()
                    else:
                        keep.append(item)
                filler[:] = keep

            def emit_proj(sbk, enqueue):
                """qkv projection for s-block sbk; enqueue=True drips the
                matmul groups through the filler queue so they pack into
                attention's ACT-bound gaps."""
                s0 = 512 * sbk
                xt = [xpool.tile([P, 512], bf16, tag=f"xt{ch}", name=f"xt{sbk}_{ch}")
                      for ch in range(NCH)]
                for ch in range(NCH):
                    # gpsimd = software DGE queue; keeps DMA issue off the
                    # scalar engine whose cycles the softmax Exp needs
                    nc.gpsimd.dma_start(xt[ch][:], x3[:, ch, s0:s0 + 512])
                    if sbk == 0:
                        weng = nc.sync if ch % 2 == 0 else nc.scalar
                        weng.dma_start(w_t[ch][:], w3[:, ch])

                def qk_group(ft, sbk=sbk, xt=xt):
                    qkps = psum.tile([P, 512], f32, tag="acc", bufs=2,
                                     name=f"qkps{sbk}_{ft}")
                    for ch in range(NCH):
                        nc.tensor.matmul(
                            qkps[:],
                            lhsT=w_t[ch][:, 128 * ft:128 * (ft + 1)],
                            rhs=xt[ch][:],
                            start=(ch == 0), stop=(ch == NCH - 1),
                            skip_group_check=True,
                        )
                    nc.any.tensor_copy(qk_t[sbk][ft % 2][:, ft // 2, :],
                                       qkps[:])

                def v_group(st, sbk=sbk, xt=xt):
                    vps = psum.tile([P, 256], f32, tag="acc", bufs=2,
                                    name=f"vps{sbk}_{st}")
                    for ch in range(NCH):
                        nc.tensor.matmul(
                            vps[:],
                            lhsT=xt[ch][:, 128 * st:128 * (st + 1)],
                            rhs=w_t[ch][:, 512:768],
                            start=(ch == 0), stop=(ch == NCH - 1),
                            skip_group_check=True,
                        )
                    nc.any.tensor_copy(
                        v4[sbk][st][:, :, 0:64],
                        vps.rearrange("p (h e) -> p h e", h=4),
                    )

                # pair-0 q/k first so attention(sbk, hp=0) unblocks early
                order = [(qk_group, 0), (qk_group, 2), (v_group, 0), (v_group, 1),
                         (v_group, 2), (v_group, 3), (qk_group, 1), (qk_group, 3)]
                for fn, i in order:
                    if enqueue:
                        # proj for s-block sbk must be fully emitted before
                        # attention(ib=sbk) scores: deadline = sbk - 1
                        cost = 1710 if fn is qk_group else 855
                        filler.append((cost, sbk - 1, lambda fn=fn, i=i: fn(i)))
                    else:
                        fn(i)

            def enqueue_normalize(ib, rs_ib):
                def recip_item(ib=ib, rs_ib=rs_ib):
                    rs_inv = small.tile([97, 512], f32, tag="rsi", name=f"rsi{ib}")
                    nc.vector.reciprocal_approx_fast(rs_inv[:], rs_ib[:])
                    rs_inv_b = small.tile([97, 512], bf16, tag="rsib",
                                          name=f"rsib{ib}")
                    nc.vector.tensor_copy(rs_inv_b[:], rs_inv[:])
                    _state[ib] = rs_inv_b

                def norm_head(h, ib=ib):
                    rs_inv = _state[ib]
                    po = 64 * (h % 2)
                    hp = h // 2
                    bcps = psum.tile([64, 512], f32, tag="acc", bufs=2,
                                     name=f"bcps{h}_{ib}")
                    nc.tensor.matmul(
                        bcps[:], lhsT=ones97[32 * h:32 * h + 1, :],
                        rhs=rs_inv[32 * h:32 * h + 1, :],
                        start=True, stop=True,
                        skip_group_check=True,
                        tile_position=(32 * h, 0),
                    )
                    nc.vector.tensor_mul(
                        at_t[ib][po:po + 64, hp, :],
                        at_t[ib][po:po + 64, hp, :], bcps[:]
                    )

                # rs pool bufs=2: recip(ib) must emit before rs(ib+2) memset
                filler.append((100, ib + 1, recip_item))
                for h in range(4):
                    filler.append((215, ib + 1, lambda h=h: norm_head(h)))

            def enqueue_outproj(ib):
                def op_item(its, ec, ib=ib):
                    it = 4 * ib + its
                    key = ("ot", it)
                    if ec == 0:
                        _state[key] = outpool.tile([P, E], bf16, tag="ot",
                                                   name=f"ot{it}")
                    ot = _state[key]
                    ops = psum.tile([P, 512], f32, tag="acc", bufs=2,
                                    name=f"ops{it}_{ec}")
                    for c in range(2):
                        nc.tensor.matmul(
                            ops[:],
                            lhsT=at_t[ib][:, c, 128 * its:128 * (its + 1)],
                            rhs=wo_sb[:, c, 512 * ec:512 * (ec + 1)],
                            start=(c == 0), stop=(c == 1),
                            skip_group_check=True,
                        )
                    nc.vector.tensor_copy(ot[:, 512 * ec:512 * (ec + 1)],
                                          ops[:])
                    if ec == 1:
                        nc.sync.dma_start(outd[128 * it:128 * (it + 1), :], ot[:])

                for its in range(4):
                    for ec in range(2):
                        filler.append((427, 99, lambda its=its, ec=ec: op_item(its, ec)))

            _state = {}
            emit_proj(0, enqueue=False)
            nc.sync.dma_start(mask_sb[:, 0, :], maskd[:])
            nc.sync.dma_start(wo_sb[:], wo3[:])

            # ---- attention: (ib, head-pair) sweeps, software-pipelined.
            # Per j-tile: paired scores (64x128 row tiles T0+T8 run
            # concurrently), one Exp evacuating both heads, causal mask mul
            # on the diagonal, then AV accumulation per head.  The AV of
            # j-tile pair k runs while ACT computes exp of pair k+1.
            def emit_scores(ib, hp, bat, pb2):
                for jj in range(2):
                    jt = 2 * bat + jj
                    t = jt - 4 * ib
                    e0 = 128 * t if t >= 0 else 0
                    sc = psum.tile([P, 2, 512], f32, tag="sc", bufs=2,
                                   name=f"sc{ib}_{hp}_{jt}")
                    pb = probs.tile([P, 2, 512], bf16, tag="pb",
                                    name=f"pb{ib}_{hp}_{jt}")
                    pb2[jj] = pb
                    lt = jt % 4
                    sb = jt // 4
                    for par in range(2):
                        nc.tensor.matmul(
                            sc[:, par, e0:],
                            lhsT=qk_t[sb][hp][64 * par:64 * par + 64, 1,
                                              128 * lt:128 * (lt + 1)],
                            rhs=qk_t[ib][hp][64 * par:64 * par + 64, 0, e0:],
                            start=True, stop=True, skip_group_check=True,
                        )
                    # probs = exp(scores / sqrt(D)); no max-subtraction
                    nc.scalar.activation(pb[:, :, e0:], sc[:, :, e0:],
                                         Exp, scale=0.125)
                    if t >= 0:
                        # causal triangle on the partially-masked 128 columns
                        nc.vector.tensor_mul(
                            pb[:, :, e0:e0 + 128], pb[:, :, e0:e0 + 128],
                            mask_sb[:, 0:1, :].to_broadcast((P, 2, 128)),
                        )

            def emit_av(ib, hp, bat, pb2, atp2, njt):
                for jj in range(2):
                    jt = 2 * bat + jj
                    t = jt - 4 * ib
                    c0 = 128 * t if t > 0 else 0
                    for par in range(2):
                        nc.tensor.matmul(
                            atp2[par][:, c0:],
                            lhsT=v4[jt // 4][jt % 4][:, 2 * hp + par, :],
                            rhs=pb2[jj][:, par, c0:],
                            start=(jt == 0), stop=(jt == njt - 1),
                            skip_group_check=True,
                        )

            def evac_pair(ib, hp, atp2, rs_ib):
                for par in range(2):
                    h = 2 * hp + par
                    nc.vector.tensor_copy(rs_ib[32 * h:32 * h + 1, :],
                                          atp2[par][64:65, :])
                    nc.any.tensor_copy(
                        at_t[ib][64 * par:64 * par + 64, hp, :],
                        atp2[par][0:64, :],
                    )

            # per-batch filler PE budget: total deferred PE work (~48us)
            # spread over the 40 j-tile-pair batches
            BAT_BUDGET = 1000

            pend = None  # (ib, hp, bat, pb2, atp2, njt) awaiting AV
            for ib in range(NIB):
                if ib + 1 < NSB:
                    emit_proj(ib + 1, enqueue=True)
                rs_ib = small.tile([97, 512], f32, tag="rs", name=f"rs{ib}")
                nc.vector.memset(rs_ib[:], 1.0)
                _state[("rs", ib)] = rs_ib
                for hp in range(2):
                    njt = 4 * (ib + 1)
                    atp2 = [psum.tile([65, 512], f32, tag="at", bufs=2,
                                      name=f"atps{ib}_{hp}_{par}")
                            for par in range(2)]
                    for bat in range(njt // 2):
                        pb2 = [None, None]
                        emit_scores(ib, hp, bat, pb2)
                        if pend is not None:
                            emit_av(*pend)
                            if pend[0] != ib or pend[1] != hp:
                                # previous pair finished: evacuate + enqueue
                                p_ib, p_hp = pend[0], pend[1]
                                evac_pair(p_ib, p_hp, pend[4], _state[("rs", p_ib)])
                                if p_hp == 1:
                                    enqueue_normalize(p_ib, _state[("rs", p_ib)])
                                    enqueue_outproj(p_ib)
                        pend = (ib, hp, bat, pb2, atp2, njt)
                        drain_budget(BAT_BUDGET)
                flush_due(ib)
            # tail
            emit_av(*pend)
            evac_pair(pend[0], pend[1], pend[4], _state[("rs", pend[0])])
            enqueue_normalize(pend[0], _state[("rs", pend[0])])
            enqueue_outproj(pend[0])
            while filler:
                filler.pop(0)[2]()

    nc.compile()
    return nc


def _get_program(S=_S):
    if S not in _PROGRAM_CACHE:
        _PROGRAM_CACHE[S] = _build_program(S)
    return _PROGRAM_CACHE[S]


def _make_mask():
    import ml_dtypes
    pp = np.arange(_P)[:, None]
    cc = np.arange(128)[None, :]
    return (cc >= pp).astype(ml_dtypes.bfloat16)


def make_in_maps(x, w_qkv, w_out):
    import ml_dtypes
    bf16 = ml_dtypes.bfloat16
    x = np.asarray(x, np.float32)
    w_qkv = np.asarray(w_qkv, np.float32)
    w_out = np.asarray(w_out, np.float32)
    E = _E
    mask = _make_mask()
    xTs = [np.ascontiguousarray(x[b].T).astype(bf16) for b in range(_B)]
    wqs, wos = [], []
    for g in range(4):
        W = np.concatenate(
            [
                w_qkv[256 * g:256 * (g + 1)],
                w_qkv[E + 256 * g:E + 256 * (g + 1)],
                w_qkv[2 * E + 256 * g:2 * E + 256 * (g + 1)],
            ],
            axis=0,
        )  # [768, E]
        wqs.append(np.ascontiguousarray(W.T).astype(bf16))          # [E, 768]
        wos.append(np.ascontiguousarray(
            w_out[:, 256 * g:256 * (g + 1)].T).astype(bf16))        # [256, E]
    in_maps = []
    for core in range(8):
        b, g = core // 4, core % 4
        in_maps.append(
            {"xT": xTs[b], "wqkvT": wqs[g], "woutT": wos[g], "mask": mask}
        )
    return in_maps


LAST_TRACE_DIR = None


def _enable_jax_compile_cache():
    try:
        import jax

        jax.config.update("jax_compilation_cache_dir", "/tmp/jax_cache")
        jax.config.update("jax_persistent_cache_min_compile_time_secs", 0.0)
        jax.config.update("jax_persistent_cache_min_entry_size_bytes", -1)
    except Exception:
        pass


def kernel(x, w_qkv, w_out, b_out):
    global LAST_EXEC_TIME_NS, LAST_TRACE_DIR
    from concourse.bass_utils import run_bass_kernel_spmd

    _enable_jax_compile_cache()
    b_out = np.asarray(b_out, np.float32)
    in_maps = make_in_maps(x, w_qkv, w_out)
    nc = _get_program()
    trace = bool(int(os.environ.get("BASS_PROFILE", "0")))
    tmpdir = None
    if trace:
        import tempfile

        tmpdir = tempfile.mkdtemp(prefix="bass_trace_")
        LAST_TRACE_DIR = tmpdir
    res = run_bass_kernel_spmd(
        nc, in_maps, core_ids=list(range(8)), trace=trace, tmpdir=tmpdir
    )
    LAST_EXEC_TIME_NS = res.exec_time_ns
    out = np.zeros((_B, _S, _E), np.float32)
    for core in range(8):
        out[core // 4] += np.asarray(res.results[core]["out"], np.float32)
    out += b_out[None, None, :]
    return out
